# revision 23
# baseline (speedup 1.0000x reference)
"""BitNet MNIST MLP forward on 8 Trainium2 NeuronCores (pure data parallel).

Reference math (per _bitlinear): out = (x/sx) @ w_q.T * sx with per-row
sx = max(|x|) -- the activation scale cancels exactly, so we compute
x @ w_q.T directly.  Ternary w_q is precomputed on host (exact in bf16).

Per-core dataflow (batch shard 8192 rows, chunks of 512 batch columns):
  activations live feature-major [feat_part(128) x batch_free] in SBUF, so
  every layer's matmul contracts features on partitions with stationary
  (pre-transposed) weights and NO on-chip transposes.

v2 vs v1 (516us -> ~478us):
  - L1 contracts exactly 784 rows: 6 full 128-row k-tiles + one 16-row
    tail packed as 4 CONCURRENT row-strip matmuls (tile_position=(32g,0),
    weights+x replicated at partitions 32g..32g+15, thin matmuls LEAD each
    accumulation group).  Saves 6 of 56 PE slots per chunk (~20us total)
    vs zero-padding to 896.
  - RMS ssq ones-matmuls trail their DVE reduce tree by >=6us of
    independent PE work (L1 halves / L2 oi-groups interleaved), so PE
    never stalls on the tree.  ssq tiles live in the same PSUM pool as
    the mm tiles: 18 allocs/superstep = 3*bufs keeps the bank mapping
    phase-stable (every buf's previous user frees >=4us earlier).
  - gelu emission is always placed AFTER the PSUM-draining copies of the
    next mm group so the in-order ACT queue never delays a copy (and
    with it the reduce tree) behind a 3.7us gelu.
  - hs/gelu stay whole on DVE/ACT: offloading to GpSimd was measured to
    slow concurrent DVE ops 2.4x (shared SBUF path) -- net loss.
  - x chunks prefetched one superstep ahead; chunk 0 loads thin-tail
    tiles first, then (x k-tile, w1 k-tile) pairs, so the first matmul
    starts ~10us in instead of ~16us.
  rsqrt = int bit-trick seed + 1 Newton iteration on DVE (no ACT table
  thrash; ScalarE only runs {copy, gelu} = one table set).
"""

from contextlib import ExitStack

import numpy as np
import ml_dtypes

import concourse.bacc as bacc
import concourse.bass as bass
import concourse.mybir as mybir
import concourse.tile as tile
from concourse.bass_utils import run_bass_kernel_spmd

N_CORES = 8
B, IN, H, OUT = 65536, 784, 1024, 10
BPC = B // N_CORES   # 8192 rows per core
KF = 6               # full 128-row contraction tiles, layer 1 (768 rows)
KT = IN - KF * 128   # 16-row thin tail
K2 = H // 128        # 8 contraction tiles, layers 2/3
HO = H // 128        # 8 output-feature tiles
BS = 512             # batch columns per chunk
NB = BPC // BS       # 16 chunks
EPS_Q = 1e-5
MAGIC = 0x5F3759DF

F32 = mybir.dt.float32
BF16 = mybir.dt.bfloat16
I32 = mybir.dt.int32
ALU = mybir.AluOpType
ACTF = mybir.ActivationFunctionType

_cache = {}
LAST_RESULTS = None  # test.py reads exec_time_ns off this


def _build(g_is_one=True):
    # Bacc (not raw Bass): its compile() runs generate_event_semaphores(),
    # which splits multi-wait sync_infos down to the 1-wait HW limit.
    nc = bacc.Bacc("TRN2", target_bir_lowering=False, debug=False, num_devices=N_CORES)

    xt = nc.dram_tensor("xt", [IN, BPC], BF16, kind="ExternalInput").ap()
    w1t = nc.dram_tensor("w1t", [KF * 128, H], BF16, kind="ExternalInput").ap()
    w1th = nc.dram_tensor("w1th", [128, H], BF16, kind="ExternalInput").ap()
    w2t = nc.dram_tensor("w2t", [H, H], BF16, kind="ExternalInput").ap()
    w3t = nc.dram_tensor("w3t", [H, OUT], BF16, kind="ExternalInput").ap()
    g1 = nc.dram_tensor("g1", [128, HO], F32, kind="ExternalInput").ap()
    g2 = nc.dram_tensor("g2", [128, HO], F32, kind="ExternalInput").ap()
    outt = nc.dram_tensor("outt", [OUT, BPC], F32, kind="ExternalOutput").ap()

    with tile.TileContext(nc) as tc, ExitStack() as ctx:
        wp = ctx.enter_context(tc.tile_pool(name="weights", bufs=1))
        xp = ctx.enter_context(tc.tile_pool(name="x", bufs=3))
        hp = ctx.enter_context(tc.tile_pool(name="h", bufs=2))
        # gelu outputs cross one pipeline stage (written superstep s, read s+1)
        hq = ctx.enter_context(tc.tile_pool(name="hout", bufs=3))
        rp = ctx.enter_context(tc.tile_pool(name="rsq", bufs=2))
        op = ctx.enter_context(tc.tile_pool(name="out", bufs=3))
        pp = ctx.enter_context(tc.tile_pool(name="ps", bufs=6, space="PSUM"))
        sp = ctx.enter_context(tc.tile_pool(name="ssq", bufs=1, space="PSUM"))
        p3 = ctx.enter_context(tc.tile_pool(name="ps3", bufs=1, space="PSUM"))

        # --- resident weights ---
        w1sb = wp.tile([128, KF, H], BF16)
        w1tsb = wp.tile([128, H], BF16)   # 16-row tail replicated at 0/32/64/96
        g1sb = wp.tile([128, HO], F32)
        ones = wp.tile([128, 128], BF16)
        nc.vector.memset(ones[:], 1.0 / H)
        w2sb = wp.tile([128, K2, H], BF16)
        w3sb = wp.tile([128, K2, OUT], BF16)
        g2sb = wp.tile([128, HO], F32)

        xt_r = xt[0 : KF * 128, :].rearrange("(k p) b -> p k b", p=128)

        def dma_x(s, xsb, xth):
            # thin-tail tiles first: the 4-way packed tail matmuls LEAD each
            # half's accumulation groups, so xth/w1th must land first.
            bsl = slice(s * BS, (s + 1) * BS)
            if s == 0:
                # per-strip so the 4 transfers ride 4 queues in parallel
                for g in range(4):
                    nc.sync.dma_start(
                        w1tsb[32 * g : 32 * g + KT, :], w1th[32 * g : 32 * g + KT, :]
                    )
            for g in range(4):
                nc.sync.dma_start(
                    xth[32 * g : 32 * g + KT, :], xt[KF * 128 : IN, bsl]
                )
            for k in range(KF):
                nc.sync.dma_start(xsb[:, k, :], xt_r[:, k, bsl])
                if s == 0:
                    nc.sync.dma_start(
                        w1sb[:, k, :], w1t[k * 128 : (k + 1) * 128, :]
                    )
            if s == 0:
                nc.sync.dma_start(g1sb[:], g1[:])

        def load_l23_weights():
            for k in range(K2):
                nc.sync.dma_start(w2sb[:, k, :], w2t[k * 128 : (k + 1) * 128, :])
            for k in range(K2):
                nc.sync.dma_start(w3sb[:, k, :], w3t[k * 128 : (k + 1) * 128, :])
            nc.sync.dma_start(g2sb[:], g2[:])

        # ---- layer-1 matmul phase: one half (4 oi tiles) ------------------
        # 16-row tail FIRST as 4 concurrent row-strip matmuls (start=True on
        # all 4 banks), then oi-major mains (copies pipeline per-oi on ACT
        # during the phase), then the batched half square/reduce on DVE.
        def l1_phase(xsb, xth, ps_tiles, st, half):
            ois = range(4 * half, 4 * half + 4)
            for j, oi in enumerate(ois):
                nc.tensor.matmul(
                    ps_tiles[j][:],
                    lhsT=w1tsb[32 * j : 32 * j + KT, oi * 128 : (oi + 1) * 128],
                    rhs=xth[32 * j : 32 * j + KT, :],
                    start=True,
                    stop=False,
                    tile_position=(32 * j, 0),
                )
            hraw, hsq, pairs, quads = st["hraw"], st["hsq"], st["pairs"], st["quads"]
            for j, oi in enumerate(ois):
                for k in range(KF):
                    nc.tensor.matmul(
                        ps_tiles[j][:],
                        lhsT=w1sb[:, k, oi * 128 : (oi + 1) * 128],
                        rhs=xsb[:, k, :],
                        start=False,
                        stop=(k == KF - 1),
                    )
                nc.scalar.copy(hraw[:, oi, :], ps_tiles[j][:])
            h4 = slice(4 * half, 4 * half + 4)
            nc.vector.tensor_mul(hsq[:, h4, :], hraw[:, h4, :], hraw[:, h4, :])
            ev = hsq[:, h4, :].rearrange("p (j two) f -> p two j f", two=2)
            nc.vector.tensor_add(pairs[:, 2 * half : 2 * half + 2, :], ev[:, 0], ev[:, 1])
            nc.vector.tensor_add(
                quads[:, half, :], pairs[:, 2 * half, :], pairs[:, 2 * half + 1, :]
            )

        # ---- layer-2 matmul for a range of oi tiles -----------------------
        def l2_mms(rhs, ps_tiles, ois):
            for j, oi in enumerate(ois):
                ps = ps_tiles[j]
                for k in range(K2):
                    nc.tensor.matmul(
                        ps[:],
                        lhsT=w2sb[:, k, oi * 128 : (oi + 1) * 128],
                        rhs=rhs[:, k, :],
                        start=(k == 0),
                        stop=(k == K2 - 1),
                    )

        def l2_copies(ps_tiles, st, ois, fine=False):
            hraw, hsq, pairs = st["hraw"], st["hsq"], st["pairs"]
            for j, oi in enumerate(ois):
                nc.scalar.copy(hraw[:, oi, :], ps_tiles[j][:])
                if fine:
                    nc.vector.tensor_mul(hsq[:, oi, :], hraw[:, oi, :], hraw[:, oi, :])
                    if oi % 2 == 1:
                        nc.vector.tensor_add(
                            pairs[:, oi // 2, :], hsq[:, oi - 1, :], hsq[:, oi, :]
                        )

        def l2_tree_half(st, half, fine=False):
            hraw, hsq, pairs, quads = st["hraw"], st["hsq"], st["pairs"], st["quads"]
            if not fine:
                h4 = slice(4 * half, 4 * half + 4)
                nc.vector.tensor_mul(hsq[:, h4, :], hraw[:, h4, :], hraw[:, h4, :])
                ev = hsq[:, h4, :].rearrange("p (j two) f -> p two j f", two=2)
                nc.vector.tensor_add(
                    pairs[:, 2 * half : 2 * half + 2, :], ev[:, 0], ev[:, 1]
                )
            nc.vector.tensor_add(
                quads[:, half, :], pairs[:, 2 * half, :], pairs[:, 2 * half + 1, :]
            )

        def mk_state(tag):
            # hraw/quads of the L2 state cross a superstep boundary (3 bufs);
            # hsq/pairs/octs die within their superstep (shared tags, 2 bufs).
            nb = 3 if tag == "2" else 2
            return {
                "hraw": hp.tile([128, HO, BS], BF16, tag=f"hraw{tag}", name=f"hraw{tag}", bufs=nb),
                "hsq": hp.tile([128, HO, BS], BF16, tag="hsq", name=f"hsq{tag}", bufs=2),
                "pairs": hp.tile([128, 4, BS], BF16, tag="hsqp", name=f"hsqp{tag}", bufs=2),
                "quads": hp.tile([128, 2, BS], BF16, tag=f"hsqq{tag}", name=f"hsqq{tag}", bufs=nb),
                "octs": hp.tile([128, BS], BF16, tag="hsqo", name=f"hsqo{tag}", bufs=2),
            }

        def finish_tree(st):
            nc.vector.tensor_add(st["octs"][:], st["quads"][:, 0, :], st["quads"][:, 1, :])

        def ssq_mm(st):
            # ssq lives in the same PSUM pool/tag as the mm tiles so the
            # per-superstep allocation count stays 18 = 3*bufs (stable
            # buf mapping; every buf's previous user frees >=4us earlier).
            ssq = pp.tile([128, BS], F32, tag="mm", name="ssq")
            nc.tensor.matmul(ssq[:], lhsT=ones[:], rhs=st["octs"][:], start=True, stop=True)
            return ssq

        def rsqrt_chain(ssq):
            ti = rp.tile([128, BS], I32, tag="ti")
            nc.vector.tensor_scalar(
                ti[:], ssq[:].bitcast(I32), 1, -1,
                op0=ALU.arith_shift_right, op1=ALU.bitwise_xor,
            )  # ~(v >> 1)
            y0 = rp.tile([128, BS], I32, tag="y0")
            nc.vector.tensor_scalar(y0[:], ti[:], MAGIC + 1, None, op0=ALU.add)
            y0f = y0[:].bitcast(F32)
            t1 = rp.tile([128, BS], F32, tag="t1")
            nc.vector.tensor_mul(t1[:], y0f, y0f)
            t2 = rp.tile([128, BS], F32, tag="t2")
            nc.vector.tensor_mul(t2[:], t1[:], ssq[:])
            nc.vector.tensor_scalar(t2[:], t2[:], -0.5, 1.5, op0=ALU.mult, op1=ALU.add)
            rinv = rp.tile([128, BS], BF16, tag="rinv")
            nc.vector.tensor_mul(rinv[:], y0f, t2[:])
            return rinv

        def chain_hs(st, ssq):
            """rsqrt chain then hs = hraw*rinv, all on DVE.  (GpSimd tensor
            ops were tried for the second half: co-running them slows DVE
            2.4x -- shared SBUF path -- so GpSimd stays idle.)"""
            rinv = rsqrt_chain(ssq)
            rb = rinv[:].rearrange("p (o f) -> p o f", o=1)
            hs = hp.tile([128, HO, BS], BF16, tag="hs", name="hs", bufs=2)
            nc.vector.tensor_mul(
                hs[:], st["hraw"][:], rb.broadcast_to([128, HO, BS])
            )
            return hs

        def gelu_emit(hs, g_sb):
            hout = hq.tile([128, HO, BS], BF16, tag="hout", name="hout")
            if g_is_one:
                nc.scalar.activation(hout[:], hs[:], ACTF.Gelu)
            else:
                for oi in range(HO):
                    nc.scalar.activation(
                        hout[:, oi, :], hs[:, oi, :], ACTF.Gelu,
                        scale=g_sb[:, oi : oi + 1],
                    )
            return hout

        def norm_gelu_fine(st, ssq, g_sb):
            """Tail chunk: per-oi hs+gelu (all DVE) to minimize latency into
            the L3 strip matmuls."""
            hraw = st["hraw"]
            rinv = rsqrt_chain(ssq)
            hs = hp.tile([128, HO, BS], BF16, tag="hs", name="hs", bufs=2)
            hout = hq.tile([128, HO, BS], BF16, tag="hout", name="hout")
            for oi in range(HO):
                nc.vector.tensor_mul(hs[:, oi, :], hraw[:, oi, :], rinv[:])
                if g_is_one:
                    nc.scalar.activation(hout[:, oi, :], hs[:, oi, :], ACTF.Gelu)
                else:
                    nc.scalar.activation(
                        hout[:, oi, :], hs[:, oi, :], ACTF.Gelu,
                        scale=g_sb[:, oi : oi + 1],
                    )
            return hout

        def l3_out(c, h2):
            """L3 (M=10): 4 col-strips of the PE array concurrently, 2
            K-chunks accumulated per strip; strips merge on ACT+DVE."""
            ps3 = p3.tile([128, BS], F32, tag="mm3")
            for g in range(4):
                for kk in range(2):
                    k = 2 * g + kk
                    nc.tensor.matmul(
                        ps3[32 * g : 32 * g + OUT, :],
                        lhsT=w3sb[:, k, :],
                        rhs=h2[:, k, :],
                        start=(kk == 0),
                        stop=(kk == 1),
                        tile_position=(0, 32 * g),
                    )
            osb = op.tile([OUT, BS], F32, tag="osb")
            nc.scalar.copy(osb[:], ps3[0:OUT, :])
            for g in range(1, 4):
                nc.vector.tensor_add(osb[:], osb[:], ps3[32 * g : 32 * g + OUT, :])
            nc.sync.dma_start(outt[:, c * BS : (c + 1) * BS], osb[:])

        # ------------------------------------------------------------------
        # Superstep s emission (PE program order):
        #   L1(s) half A  ->  ssq(L2(s-2))  ->  L1(s) half B  ->
        #   L2(s-1) oi0..1  ->  ssq(L1(s))  ->  L2(s-1) oi2..7  ->  L3(s-2)
        # Both ssq ones-matmuls trail their reduce tree by >=3us of
        # independent PE work, so PE never stalls on DVE.
        # gelu(L2(s-2)) is emitted AFTER half B so the ACT queue never puts
        # a gelu in front of the PSUM-draining copies that gate the tree.
        # ------------------------------------------------------------------
        xtiles = {}
        l2st = {}   # chunk -> tree state for L2
        h1s = {}    # chunk -> gelu(L1) output
        h2s = {}    # chunk -> gelu(L2) output

        def alloc_x():
            xsb = xp.tile([128, KF, BS], BF16, tag="xsb", name="xsb")
            xth = xp.tile([128, BS], BF16, tag="xth", name="xth")
            return xsb, xth

        xtiles[0] = alloc_x()
        dma_x(0, *xtiles[0])

        for s in range(NB + 2):
            st2o = l2st.pop(s - 2, None)
            # --- L1(s) half A ---
            if s < NB:
                xsb, xth = xtiles.pop(s)
                st1 = mk_state("1")
                psA = [pp.tile([128, BS], F32, tag="mm", name=f"psA{i}") for i in range(4)]
                l1_phase(xsb, xth, psA, st1, 0)
            # prefetch x(s+1) (before w2/w3 at s=0 so chunk 1 isn't queued
            # behind 2MB of layer-2 weights)
            if s + 1 < NB:
                xtiles[s + 1] = alloc_x()
                dma_x(s + 1, *xtiles[s + 1])
            if s == 0:
                load_l23_weights()
            # --- L1(s) half B ---
            if s < NB:
                psB = [pp.tile([128, BS], F32, tag="mm", name=f"psB{i}") for i in range(4)]
                l1_phase(xsb, xth, psB, st1, 1)
                finish_tree(st1)
            # --- ssq matmul for L2(s-2), after BOTH L1 halves so the PE
            # slot trails the (late-spilling) L2 tree by ~11us; the rsqrt
            # chain is emitted after ps01 so treeB keeps DVE priority ---
            hs2 = None
            ssq2 = None
            if st2o is not None and s < NB:
                ssq2 = ssq_mm(st2o)
            # --- L2(s-1) oi0..1 ---
            if 1 <= s <= NB:
                c = s - 1
                h1 = h1s.pop(c)
                st2 = mk_state("2")
                l2st[c] = st2
                fine2 = c == NB - 1
                ps01 = [pp.tile([128, BS], F32, tag="mm", name=f"ps01_{i}") for i in range(2)]
                l2_mms(h1, ps01, [0, 1])
                l2_copies(ps01, st2, [0, 1], fine=fine2)
            # at s>=NB there is no L1 phase: the ssq2 slot goes behind the
            # ps01 matmuls instead (which are ready early thanks to the
            # s==NB-1 early gelu1 below)
            if st2o is not None and ssq2 is None:
                ssq2 = ssq_mm(st2o)
            # --- rsqrt/hs + gelu for L2(s-2): gelu after the ps01 copies so
            # the ACT queue never has a 3.7us gelu in front of PSUM copies ---
            if ssq2 is not None:
                if (s - 2) == NB - 1:
                    h2s[s - 2] = norm_gelu_fine(st2o, ssq2, g2sb)
                else:
                    hs2 = chain_hs(st2o, ssq2)
                    h2s[s - 2] = gelu_emit(hs2, g2sb)
            # --- L2(s-1) oi2..3 ---
            if 1 <= s <= NB:
                ps23 = [pp.tile([128, BS], F32, tag="mm", name=f"ps23_{i}") for i in range(2)]
                l2_mms(h1, ps23, [2, 3])
                l2_copies(ps23, st2, [2, 3], fine=fine2)
                l2_tree_half(st2, 0, fine=fine2)
            # --- ssq + rsqrt/hs for L1(s): 6.8us of L2 work ahead of it so
            # the ones-matmul never waits on the L1 tree; gelu1 after psC ---
            hs1 = None
            if s < NB:
                ssq1 = ssq_mm(st1)
                hs1 = chain_hs(st1, ssq1)
            # --- L2(s-1) oi4..7 ---
            if 1 <= s <= NB:
                psC = [pp.tile([128, BS], F32, tag="mm", name=f"psC{i}") for i in range(4)]
                l2_mms(h1, psC, [4, 5, 6, 7])
                l2_copies(psC, st2, [4, 5, 6, 7], fine=fine2)
                l2_tree_half(st2, 1, fine=fine2)
                finish_tree(st2)
            # --- gelu for L1(s) ---
            if hs1 is not None:
                h1s[s] = gelu_emit(hs1, g1sb)
            # --- L3(s-2) ---
            if s >= 2 and (s - 2) in h2s:
                l3_out(s - 2, h2s.pop(s - 2))

    nc.compile()
    return nc


def _quant(w):
    s = max(float(np.mean(np.abs(w))), EPS_Q)
    return np.clip(np.round(w / s), -1.0, 1.0)


def kernel(x, w1, g1, w2, g2, w3):
    global LAST_RESULTS
    bf = ml_dtypes.bfloat16

    w1q = _quant(np.asarray(w1, np.float32))  # [H, IN]
    w2q = _quant(np.asarray(w2, np.float32))  # [H, H]
    w3q = _quant(np.asarray(w3, np.float32))  # [OUT, H]

    w1T = w1q.T.astype(bf)                    # [IN, H]
    w1t_np = np.ascontiguousarray(w1T[: KF * 128])
    w1th_np = np.zeros([128, H], dtype=bf)    # 16-row tail at 0/32/64/96
    for g in range(4):
        w1th_np[32 * g : 32 * g + KT] = w1T[KF * 128 :]
    w2t_np = np.ascontiguousarray(w2q.T.astype(bf))
    w3t_np = np.ascontiguousarray(w3q.T.astype(bf))
    g1_np = np.ascontiguousarray(np.asarray(g1, np.float32).reshape(HO, 128).T)
    g2_np = np.ascontiguousarray(np.asarray(g2, np.float32).reshape(HO, 128).T)

    xt_np = np.asarray(x, np.float32).T.astype(bf)  # [IN, B]

    g_is_one = bool(np.all(np.asarray(g1) == 1.0) and np.all(np.asarray(g2) == 1.0))
    key = ("nc", g_is_one)
    if key not in _cache:
        _cache[key] = _build(g_is_one)
    nc = _cache[key]

    in_maps = []
    for i in range(N_CORES):
        in_maps.append(
            {
                "xt": np.ascontiguousarray(xt_np[:, i * BPC : (i + 1) * BPC]),
                "w1t": w1t_np,
                "w1th": w1th_np,
                "w2t": w2t_np,
                "w3t": w3t_np,
                "g1": g1_np,
                "g2": g2_np,
            }
        )

    res = run_bass_kernel_spmd(nc, in_maps, core_ids=list(range(N_CORES)))
    LAST_RESULTS = res

    out = np.empty([B, OUT], dtype=np.float32)
    for i in range(N_CORES):
        out[i * BPC : (i + 1) * BPC] = res.results[i]["outt"].T
    return out


# revision 24
# speedup vs baseline: 1.0024x; 1.0024x over previous
"""BitNet MNIST MLP forward on 8 Trainium2 NeuronCores (pure data parallel).

Reference math (per _bitlinear): out = (x/sx) @ w_q.T * sx with per-row
sx = max(|x|) -- the activation scale cancels exactly, so we compute
x @ w_q.T directly.  Ternary w_q is precomputed on host (exact in bf16).

Per-core dataflow (batch shard 8192 rows, chunks of 512 batch columns):
  activations live feature-major [feat_part(128) x batch_free] in SBUF, so
  every layer's matmul contracts features on partitions with stationary
  (pre-transposed) weights and NO on-chip transposes.

v2 vs v1 (516us -> ~478us):
  - L1 contracts exactly 784 rows: 6 full 128-row k-tiles + one 16-row
    tail packed as 4 CONCURRENT row-strip matmuls (tile_position=(32g,0),
    weights+x replicated at partitions 32g..32g+15, thin matmuls LEAD each
    accumulation group).  Saves 6 of 56 PE slots per chunk (~20us total)
    vs zero-padding to 896.
  - RMS ssq ones-matmuls trail their DVE reduce tree by >=6us of
    independent PE work (L1 halves / L2 oi-groups interleaved), so PE
    never stalls on the tree.  ssq tiles live in the same PSUM pool as
    the mm tiles: 18 allocs/superstep = 3*bufs keeps the bank mapping
    phase-stable (every buf's previous user frees >=4us earlier).
  - gelu emission is always placed AFTER the PSUM-draining copies of the
    next mm group so the in-order ACT queue never delays a copy (and
    with it the reduce tree) behind a 3.7us gelu.
  - hs/gelu stay whole on DVE/ACT: offloading to GpSimd was measured to
    slow concurrent DVE ops 2.4x (shared SBUF path) -- net loss.
  - x chunks prefetched one superstep ahead; chunk 0 loads thin-tail
    tiles first, then (x k-tile, w1 k-tile) pairs, so the first matmul
    starts ~10us in instead of ~16us.
  rsqrt = int bit-trick seed + 1 Newton iteration on DVE (no ACT table
  thrash; ScalarE only runs {copy, gelu} = one table set).
"""

from contextlib import ExitStack

import numpy as np
import ml_dtypes

import concourse.bacc as bacc
import concourse.bass as bass
import concourse.mybir as mybir
import concourse.tile as tile
from concourse.bass_utils import run_bass_kernel_spmd

N_CORES = 8
B, IN, H, OUT = 65536, 784, 1024, 10
BPC = B // N_CORES   # 8192 rows per core
KF = 6               # full 128-row contraction tiles, layer 1 (768 rows)
KT = IN - KF * 128   # 16-row thin tail
K2 = H // 128        # 8 contraction tiles, layers 2/3
HO = H // 128        # 8 output-feature tiles
BS = 512             # batch columns per chunk
NB = BPC // BS       # 16 chunks
EPS_Q = 1e-5
MAGIC = 0x5F3759DF

F32 = mybir.dt.float32
BF16 = mybir.dt.bfloat16
I32 = mybir.dt.int32
ALU = mybir.AluOpType
ACTF = mybir.ActivationFunctionType

_cache = {}
LAST_RESULTS = None  # test.py reads exec_time_ns off this


def _build(g_is_one=True):
    # Bacc (not raw Bass): its compile() runs generate_event_semaphores(),
    # which splits multi-wait sync_infos down to the 1-wait HW limit.
    nc = bacc.Bacc("TRN2", target_bir_lowering=False, debug=False, num_devices=N_CORES)

    xt = nc.dram_tensor("xt", [IN, BPC], BF16, kind="ExternalInput").ap()
    w1t = nc.dram_tensor("w1t", [KF * 128, H], BF16, kind="ExternalInput").ap()
    w1th = nc.dram_tensor("w1th", [128, H], BF16, kind="ExternalInput").ap()
    w2t = nc.dram_tensor("w2t", [H, H], BF16, kind="ExternalInput").ap()
    w3t = nc.dram_tensor("w3t", [H, OUT], BF16, kind="ExternalInput").ap()
    g1 = nc.dram_tensor("g1", [128, HO], F32, kind="ExternalInput").ap()
    g2 = nc.dram_tensor("g2", [128, HO], F32, kind="ExternalInput").ap()
    outt = nc.dram_tensor("outt", [OUT, BPC], F32, kind="ExternalOutput").ap()

    with tile.TileContext(nc) as tc, ExitStack() as ctx:
        wp = ctx.enter_context(tc.tile_pool(name="weights", bufs=1))
        xp = ctx.enter_context(tc.tile_pool(name="x", bufs=3))
        hp = ctx.enter_context(tc.tile_pool(name="h", bufs=2))
        # gelu outputs cross one pipeline stage (written superstep s, read s+1)
        hq = ctx.enter_context(tc.tile_pool(name="hout", bufs=3))
        rp = ctx.enter_context(tc.tile_pool(name="rsq", bufs=2))
        op = ctx.enter_context(tc.tile_pool(name="out", bufs=3))
        pp = ctx.enter_context(tc.tile_pool(name="ps", bufs=6, space="PSUM"))
        sp = ctx.enter_context(tc.tile_pool(name="ssq", bufs=1, space="PSUM"))
        p3 = ctx.enter_context(tc.tile_pool(name="ps3", bufs=1, space="PSUM"))

        # --- resident weights ---
        w1sb = wp.tile([128, KF, H], BF16)
        w1tsb = wp.tile([128, H], BF16)   # 16-row tail replicated at 0/32/64/96
        g1sb = wp.tile([128, HO], F32)
        ones = wp.tile([128, 128], BF16)
        nc.vector.memset(ones[:], 1.0 / H)
        w2sb = wp.tile([128, K2, H], BF16)
        w3sb = wp.tile([128, K2, OUT], BF16)
        g2sb = wp.tile([128, HO], F32)

        xt_r = xt[0 : KF * 128, :].rearrange("(k p) b -> p k b", p=128)

        def dma_x(s, xsb, xth):
            # thin-tail tiles first: the 4-way packed tail matmuls LEAD each
            # half's accumulation groups, so xth/w1th must land first.
            bsl = slice(s * BS, (s + 1) * BS)
            if s == 0:
                # per-strip so the 4 transfers ride 4 queues in parallel
                for g in range(4):
                    nc.sync.dma_start(
                        w1tsb[32 * g : 32 * g + KT, :], w1th[32 * g : 32 * g + KT, :]
                    )
            for g in range(4):
                nc.sync.dma_start(
                    xth[32 * g : 32 * g + KT, :], xt[KF * 128 : IN, bsl]
                )
            for k in range(KF):
                nc.sync.dma_start(xsb[:, k, :], xt_r[:, k, bsl])
                if s == 0:
                    nc.sync.dma_start(
                        w1sb[:, k, :], w1t[k * 128 : (k + 1) * 128, :]
                    )
            if s == 0:
                nc.sync.dma_start(g1sb[:], g1[:])

        def load_l23_weights():
            for k in range(K2):
                nc.sync.dma_start(w2sb[:, k, :], w2t[k * 128 : (k + 1) * 128, :])
            for k in range(K2):
                nc.sync.dma_start(w3sb[:, k, :], w3t[k * 128 : (k + 1) * 128, :])
            nc.sync.dma_start(g2sb[:], g2[:])

        # ---- layer-1 matmul phase: one half (4 oi tiles) ------------------
        # 16-row tail FIRST as 4 concurrent row-strip matmuls (start=True on
        # all 4 banks), then oi-major mains (copies pipeline per-oi on ACT
        # during the phase), then the batched half square/reduce on DVE.
        def l1_phase(xsb, xth, ps_tiles, st, half):
            ois = range(4 * half, 4 * half + 4)
            for j, oi in enumerate(ois):
                nc.tensor.matmul(
                    ps_tiles[j][:],
                    lhsT=w1tsb[32 * j : 32 * j + KT, oi * 128 : (oi + 1) * 128],
                    rhs=xth[32 * j : 32 * j + KT, :],
                    start=True,
                    stop=False,
                    tile_position=(32 * j, 0),
                )
            hraw, hsq, pairs, quads = st["hraw"], st["hsq"], st["pairs"], st["quads"]
            for j, oi in enumerate(ois):
                for k in range(KF):
                    nc.tensor.matmul(
                        ps_tiles[j][:],
                        lhsT=w1sb[:, k, oi * 128 : (oi + 1) * 128],
                        rhs=xsb[:, k, :],
                        start=False,
                        stop=(k == KF - 1),
                    )
                nc.scalar.copy(hraw[:, oi, :], ps_tiles[j][:])
            h4 = slice(4 * half, 4 * half + 4)
            nc.vector.tensor_mul(hsq[:, h4, :], hraw[:, h4, :], hraw[:, h4, :])
            ev = hsq[:, h4, :].rearrange("p (j two) f -> p two j f", two=2)
            nc.vector.tensor_add(pairs[:, 2 * half : 2 * half + 2, :], ev[:, 0], ev[:, 1])
            nc.vector.tensor_add(
                quads[:, half, :], pairs[:, 2 * half, :], pairs[:, 2 * half + 1, :]
            )

        # ---- layer-2 matmul for a range of oi tiles -----------------------
        def l2_mms(rhs, ps_tiles, ois):
            for j, oi in enumerate(ois):
                ps = ps_tiles[j]
                for k in range(K2):
                    nc.tensor.matmul(
                        ps[:],
                        lhsT=w2sb[:, k, oi * 128 : (oi + 1) * 128],
                        rhs=rhs[:, k, :],
                        start=(k == 0),
                        stop=(k == K2 - 1),
                    )

        def l2_copies(ps_tiles, st, ois, fine=False):
            hraw, hsq, pairs = st["hraw"], st["hsq"], st["pairs"]
            for j, oi in enumerate(ois):
                nc.scalar.copy(hraw[:, oi, :], ps_tiles[j][:])
                if fine:
                    nc.vector.tensor_mul(hsq[:, oi, :], hraw[:, oi, :], hraw[:, oi, :])
                    if oi % 2 == 1:
                        nc.vector.tensor_add(
                            pairs[:, oi // 2, :], hsq[:, oi - 1, :], hsq[:, oi, :]
                        )

        def l2_tree_half(st, half, fine=False):
            hraw, hsq, pairs, quads = st["hraw"], st["hsq"], st["pairs"], st["quads"]
            if not fine:
                h4 = slice(4 * half, 4 * half + 4)
                nc.vector.tensor_mul(hsq[:, h4, :], hraw[:, h4, :], hraw[:, h4, :])
                ev = hsq[:, h4, :].rearrange("p (j two) f -> p two j f", two=2)
                nc.vector.tensor_add(
                    pairs[:, 2 * half : 2 * half + 2, :], ev[:, 0], ev[:, 1]
                )
            nc.vector.tensor_add(
                quads[:, half, :], pairs[:, 2 * half, :], pairs[:, 2 * half + 1, :]
            )

        def mk_state(tag):
            # hraw/quads of the L2 state cross a superstep boundary (3 bufs);
            # hsq/pairs/octs die within their superstep (shared tags, 2 bufs).
            nb = 3 if tag == "2" else 2
            return {
                "hraw": hp.tile([128, HO, BS], BF16, tag=f"hraw{tag}", name=f"hraw{tag}", bufs=nb),
                "hsq": hp.tile([128, HO, BS], BF16, tag="hsq", name=f"hsq{tag}", bufs=2),
                "pairs": hp.tile([128, 4, BS], BF16, tag="hsqp", name=f"hsqp{tag}", bufs=2),
                "quads": hp.tile([128, 2, BS], BF16, tag=f"hsqq{tag}", name=f"hsqq{tag}", bufs=nb),
                "octs": hp.tile([128, BS], BF16, tag="hsqo", name=f"hsqo{tag}", bufs=2),
            }

        def finish_tree(st):
            nc.vector.tensor_add(st["octs"][:], st["quads"][:, 0, :], st["quads"][:, 1, :])

        def ssq_mm(st):
            # ssq lives in the same PSUM pool/tag as the mm tiles so the
            # per-superstep allocation count stays 18 = 3*bufs (stable
            # buf mapping; every buf's previous user frees >=4us earlier).
            ssq = pp.tile([128, BS], F32, tag="mm", name="ssq")
            nc.tensor.matmul(ssq[:], lhsT=ones[:], rhs=st["octs"][:], start=True, stop=True)
            return ssq

        def rsqrt_chain(ssq):
            ti = rp.tile([128, BS], I32, tag="ti")
            nc.vector.tensor_scalar(
                ti[:], ssq[:].bitcast(I32), 1, -1,
                op0=ALU.arith_shift_right, op1=ALU.bitwise_xor,
            )  # ~(v >> 1)
            y0 = rp.tile([128, BS], I32, tag="y0")
            nc.vector.tensor_scalar(y0[:], ti[:], MAGIC + 1, None, op0=ALU.add)
            y0f = y0[:].bitcast(F32)
            t1 = rp.tile([128, BS], F32, tag="t1")
            nc.vector.tensor_mul(t1[:], y0f, y0f)
            t2 = rp.tile([128, BS], F32, tag="t2")
            nc.vector.tensor_mul(t2[:], t1[:], ssq[:])
            nc.vector.tensor_scalar(t2[:], t2[:], -0.5, 1.5, op0=ALU.mult, op1=ALU.add)
            rinv = rp.tile([128, BS], BF16, tag="rinv")
            nc.vector.tensor_mul(rinv[:], y0f, t2[:])
            return rinv

        def chain_hs(st, ssq):
            """rsqrt chain then hs = hraw*rinv, all on DVE.  (GpSimd tensor
            ops were tried for the second half: co-running them slows DVE
            2.4x -- shared SBUF path -- so GpSimd stays idle.)"""
            rinv = rsqrt_chain(ssq)
            rb = rinv[:].rearrange("p (o f) -> p o f", o=1)
            hs = hp.tile([128, HO, BS], BF16, tag="hs", name="hs", bufs=2)
            nc.vector.tensor_mul(
                hs[:], st["hraw"][:], rb.broadcast_to([128, HO, BS])
            )
            return hs

        def gelu_emit(hs, g_sb):
            hout = hq.tile([128, HO, BS], BF16, tag="hout", name="hout")
            if g_is_one:
                nc.scalar.activation(hout[:], hs[:], ACTF.Gelu)
            else:
                for oi in range(HO):
                    nc.scalar.activation(
                        hout[:, oi, :], hs[:, oi, :], ACTF.Gelu,
                        scale=g_sb[:, oi : oi + 1],
                    )
            return hout

        def norm_gelu_fine(st, ssq, g_sb):
            """Tail chunk: per-oi hs+gelu (all DVE) to minimize latency into
            the L3 strip matmuls."""
            hraw = st["hraw"]
            rinv = rsqrt_chain(ssq)
            hs = hp.tile([128, HO, BS], BF16, tag="hs", name="hs", bufs=2)
            hout = hq.tile([128, HO, BS], BF16, tag="hout", name="hout")
            for oi in range(HO):
                nc.vector.tensor_mul(hs[:, oi, :], hraw[:, oi, :], rinv[:])
                if g_is_one:
                    nc.scalar.activation(hout[:, oi, :], hs[:, oi, :], ACTF.Gelu)
                else:
                    nc.scalar.activation(
                        hout[:, oi, :], hs[:, oi, :], ACTF.Gelu,
                        scale=g_sb[:, oi : oi + 1],
                    )
            return hout

        def l3_out(c, h2):
            """L3 (M=10): 4 col-strips of the PE array concurrently, 2
            K-chunks accumulated per strip; strips merge on ACT+DVE."""
            ps3 = p3.tile([128, BS], F32, tag="mm3")
            for g in range(4):
                for kk in range(2):
                    k = 2 * g + kk
                    nc.tensor.matmul(
                        ps3[32 * g : 32 * g + OUT, :],
                        lhsT=w3sb[:, k, :],
                        rhs=h2[:, k, :],
                        start=(kk == 0),
                        stop=(kk == 1),
                        tile_position=(0, 32 * g),
                    )
            osb = op.tile([OUT, BS], F32, tag="osb")
            nc.scalar.copy(osb[:], ps3[0:OUT, :])
            for g in range(1, 4):
                nc.vector.tensor_add(osb[:], osb[:], ps3[32 * g : 32 * g + OUT, :])
            nc.sync.dma_start(outt[:, c * BS : (c + 1) * BS], osb[:])

        # ------------------------------------------------------------------
        # Superstep s emission (PE program order):
        #   L1(s) half A  ->  ssq(L2(s-2))  ->  L1(s) half B  ->
        #   L2(s-1) oi0..1  ->  ssq(L1(s))  ->  L2(s-1) oi2..7  ->  L3(s-2)
        # Both ssq ones-matmuls trail their reduce tree by >=3us of
        # independent PE work, so PE never stalls on DVE.
        # gelu(L2(s-2)) is emitted AFTER half B so the ACT queue never puts
        # a gelu in front of the PSUM-draining copies that gate the tree.
        # ------------------------------------------------------------------
        xtiles = {}
        l2st = {}   # chunk -> tree state for L2
        h1s = {}    # chunk -> gelu(L1) output
        h2s = {}    # chunk -> gelu(L2) output

        def alloc_x():
            xsb = xp.tile([128, KF, BS], BF16, tag="xsb", name="xsb")
            xth = xp.tile([128, BS], BF16, tag="xth", name="xth")
            return xsb, xth

        # HAM warm-up: ~4us of dummy matmuls on the resident ones tile while
        # the head DMAs land, so the PE clock gate is at 8/8 (2.4 GHz) when
        # the first real matmul issues instead of ramping from 1.2 GHz.
        # Uses the otherwise-unused sp pool bank; result is never read.
        warm = sp.tile([128, 128], F32, tag="warm", name="warm")
        for i in range(40):
            nc.tensor.matmul(
                warm[:], lhsT=ones[:], rhs=ones[:], start=(i == 0), stop=(i == 39)
            )

        xtiles[0] = alloc_x()
        dma_x(0, *xtiles[0])

        for s in range(NB + 2):
            st2o = l2st.pop(s - 2, None)
            # --- L1(s) half A ---
            if s < NB:
                xsb, xth = xtiles.pop(s)
                st1 = mk_state("1")
                psA = [pp.tile([128, BS], F32, tag="mm", name=f"psA{i}") for i in range(4)]
                l1_phase(xsb, xth, psA, st1, 0)
            # prefetch x(s+1) (before w2/w3 at s=0 so chunk 1 isn't queued
            # behind 2MB of layer-2 weights)
            if s + 1 < NB:
                xtiles[s + 1] = alloc_x()
                dma_x(s + 1, *xtiles[s + 1])
            if s == 0:
                load_l23_weights()
            # --- L1(s) half B ---
            if s < NB:
                psB = [pp.tile([128, BS], F32, tag="mm", name=f"psB{i}") for i in range(4)]
                l1_phase(xsb, xth, psB, st1, 1)
                finish_tree(st1)
            # --- ssq matmul for L2(s-2), after BOTH L1 halves so the PE
            # slot trails the (late-spilling) L2 tree by ~11us; the rsqrt
            # chain is emitted after ps01 so treeB keeps DVE priority ---
            hs2 = None
            ssq2 = None
            if st2o is not None and s < NB:
                ssq2 = ssq_mm(st2o)
            # --- L2(s-1) oi0..1 ---
            if 1 <= s <= NB:
                c = s - 1
                h1 = h1s.pop(c)
                st2 = mk_state("2")
                l2st[c] = st2
                fine2 = c == NB - 1
                ps01 = [pp.tile([128, BS], F32, tag="mm", name=f"ps01_{i}") for i in range(2)]
                l2_mms(h1, ps01, [0, 1])
                l2_copies(ps01, st2, [0, 1], fine=fine2)
            # at s>=NB there is no L1 phase: the ssq2 slot goes behind the
            # ps01 matmuls instead (which are ready early thanks to the
            # s==NB-1 early gelu1 below)
            if st2o is not None and ssq2 is None:
                ssq2 = ssq_mm(st2o)
            # --- rsqrt/hs + gelu for L2(s-2): gelu after the ps01 copies so
            # the ACT queue never has a 3.7us gelu in front of PSUM copies ---
            if ssq2 is not None:
                if (s - 2) == NB - 1:
                    h2s[s - 2] = norm_gelu_fine(st2o, ssq2, g2sb)
                else:
                    hs2 = chain_hs(st2o, ssq2)
                    h2s[s - 2] = gelu_emit(hs2, g2sb)
            # --- L2(s-1) oi2..3 ---
            if 1 <= s <= NB:
                ps23 = [pp.tile([128, BS], F32, tag="mm", name=f"ps23_{i}") for i in range(2)]
                l2_mms(h1, ps23, [2, 3])
                l2_copies(ps23, st2, [2, 3], fine=fine2)
                l2_tree_half(st2, 0, fine=fine2)
            # --- ssq + rsqrt/hs for L1(s): 6.8us of L2 work ahead of it so
            # the ones-matmul never waits on the L1 tree; gelu1 after psC ---
            hs1 = None
            if s < NB:
                ssq1 = ssq_mm(st1)
                hs1 = chain_hs(st1, ssq1)
            # --- L2(s-1) oi4..7 ---
            if 1 <= s <= NB:
                psC = [pp.tile([128, BS], F32, tag="mm", name=f"psC{i}") for i in range(4)]
                l2_mms(h1, psC, [4, 5, 6, 7])
                l2_copies(psC, st2, [4, 5, 6, 7], fine=fine2)
                l2_tree_half(st2, 1, fine=fine2)
                finish_tree(st2)
            # --- gelu for L1(s) ---
            if hs1 is not None:
                h1s[s] = gelu_emit(hs1, g1sb)
            # --- L3(s-2) ---
            if s >= 2 and (s - 2) in h2s:
                l3_out(s - 2, h2s.pop(s - 2))

    nc.compile()
    return nc


def _quant(w):
    s = max(float(np.mean(np.abs(w))), EPS_Q)
    return np.clip(np.round(w / s), -1.0, 1.0)


def kernel(x, w1, g1, w2, g2, w3):
    global LAST_RESULTS
    bf = ml_dtypes.bfloat16

    w1q = _quant(np.asarray(w1, np.float32))  # [H, IN]
    w2q = _quant(np.asarray(w2, np.float32))  # [H, H]
    w3q = _quant(np.asarray(w3, np.float32))  # [OUT, H]

    w1T = w1q.T.astype(bf)                    # [IN, H]
    w1t_np = np.ascontiguousarray(w1T[: KF * 128])
    w1th_np = np.zeros([128, H], dtype=bf)    # 16-row tail at 0/32/64/96
    for g in range(4):
        w1th_np[32 * g : 32 * g + KT] = w1T[KF * 128 :]
    w2t_np = np.ascontiguousarray(w2q.T.astype(bf))
    w3t_np = np.ascontiguousarray(w3q.T.astype(bf))
    g1_np = np.ascontiguousarray(np.asarray(g1, np.float32).reshape(HO, 128).T)
    g2_np = np.ascontiguousarray(np.asarray(g2, np.float32).reshape(HO, 128).T)

    xt_np = np.asarray(x, np.float32).T.astype(bf)  # [IN, B]

    g_is_one = bool(np.all(np.asarray(g1) == 1.0) and np.all(np.asarray(g2) == 1.0))
    key = ("nc", g_is_one)
    if key not in _cache:
        _cache[key] = _build(g_is_one)
    nc = _cache[key]

    in_maps = []
    for i in range(N_CORES):
        in_maps.append(
            {
                "xt": np.ascontiguousarray(xt_np[:, i * BPC : (i + 1) * BPC]),
                "w1t": w1t_np,
                "w1th": w1th_np,
                "w2t": w2t_np,
                "w3t": w3t_np,
                "g1": g1_np,
                "g2": g2_np,
            }
        )

    res = run_bass_kernel_spmd(nc, in_maps, core_ids=list(range(N_CORES)))
    LAST_RESULTS = res

    out = np.empty([B, OUT], dtype=np.float32)
    for i in range(N_CORES):
        out[i * BPC : (i + 1) * BPC] = res.results[i]["outt"].T
    return out
